# revision 12
# baseline (speedup 1.0000x reference)
"""Trainium2 Bass kernel for fused MultiHeadAttention + residual + LayerNorm.

Problem: query [4, 2048, 512] f32, H=8 heads (hd=64), fused QKV projection,
key-padding-mask softmax, attn @ V, residual add, LayerNorm over D=512.

Sharding: 8 cores = 4 batches x 2 query-halves. Each core handles one batch's
full K/V (T=2048) and 1024 query rows, so heads stay local and the output
LayerNorm needs no cross-core communication. K/V projection is duplicated
between the 2 cores sharing a batch; X^T columns are rotated per core so its
own query half sits at columns 0:Q (Q^T projects straight out of X^T, no
separate xq input, and attention is k-permutation-invariant with the mask
rotated to match).

K/Q projections and scores run in bf16 (fp32 PSUM accum) — fp8 K-proj was
measured offline at 2.6e-2 max error (score errors amplify through exp),
past the 2e-2 budget, so K/Q must stay bf16; score matmuls are 2x512-col
(a single 1024-col moving matmul fails the ISA check). The V projection,
attention weights P = exp(S/8 - ln64) and V run fp8e4 so those matmuls use
DoubleRow perf mode: one pass contracts TWO 128-row slices at the same
1 col/cycle -> half the PE time. fp8 operands are quantized on the host
straight from f32 (f32->bf16->fp8 double rounding costs 1.5x in max error).
The 1/64 P scaling keeps exp() inside e4m3 range for the 9-sigma score
tails (max raw score 71.9); the denominator (ones-column 0 of each V head
group) scales identically so the softmax ratio is unaffected.

Softmax exp is the Scalar/ACT bottleneck (128 tiles x [128,1024], ~1.07us
each on ACT / ~1.28us on DVE per the measured trace), so per head several
tiles go to DVE via the Schraudolph bit-trick
  fp8bits(exp(s)) ~= uint8(s*(8/ln2)*SCALE + 7.65 + maskbias)
(f32->uint8 convert is round-to-nearest saturating to [0,255] on HW, so the
exp underflow tail AND masked rows — addend -1e6 — clamp to +0.0; scores
never reach the bits>=120 inf/nan region). The uint8 tile is bitcast to
fp8e4 for the DoubleRow matmul. ~4% rms error on those P tiles, attenuated
~25x by the softmax-weighted average + f32 residual, keeps the total error
under the 2e-2 budget. GpSimd cannot read PSUM so it takes SBUF-only
work (memsets, identity).

Per-core flow:
  X^T [128,4,2048] bf16, W^T [128,4,1024] bf16 (host-interleaved
  [K_i|Q_i] 128-col groups; V columns live only in the fp8 operands),
  K^T [512,2048] bf16, Q^T head-major zero-padded to K=128 contraction
  V8  8 pair-tiles [128,2,H,80] fp8  (col 0 of each head group = 1.0)
  S^T [128k,1024q] f32 PSUM per (head,k-tile) -> exp (ACT fp8 out / DVE
      u8 trick) -> P pair tiles [128,2,1024] fp8
  O^T [65,1024] f32 = [1|V_h].T @dr P^T accumulated over 8 k-pairs
  bf16 copy -> PE-transpose [65,128]->[128,65], DVE reciprocal(denom),
  fused multiply-add folds the residual in per head slice; the recombine's
  accum_out collects row-sums for the LayerNorm mean
  LayerNorm: var = E[y^2]-mean^2 with ACT Square+accum, batched Sqrt, DVE
  normalize -> DMA out f32. When ln_weight==1 and ln_bias==0 (runtime
  check, true for this problem) the gamma/beta affine stage is skipped
  entirely; otherwise a second kernel variant with the affine ops compiles.

Scheduling (v2, driven by the measured per-instruction trace):
  - ~6us multicore-barrier preamble is fixed; the critical input DMAs are
    batched into single 3D issues (host pre-shapes xt/wt as [128,4,cols])
    so the first K-proj starts ~10us: sync queue carries xt in 512-col
    waves then xres/ln params; vector carries wt (critical 256-col prefix
    first) then the fp8 V operands; gpsimd carries the packed
    btr|maska|maskd tile then memsets; scalar carries NO DMA — it opens
    with a tiny dummy Exp (hoists the ACT table load into the DMA wait)
    then does the block-0 K/Q PSUM->SBUF copies (Identity+bias) while the
    exp stream hasn't started, then V-proj casts (alternating with DVE).
  - block-0 emission interleaves kt chunks / qt chunks / head-0 score
    tiles so the first exp fires right after the first two score matmuls.
  - every later attention@V interleaves per k-pair with the next head's
    score tiles and deferred projection chunks (block b complete before
    av(2b-1)).
  - the last head runs attention@V in 4 chunks of 256 query columns;
    after each chunk its 2 q-tiles run transpose/recombine/Square and a
    per-2-tile LayerNorm group, so only ~1/4 of the epilogue remains
    after the final matmul (the v1 two-group epilogue left a ~28us
    serial tail).
"""

import numpy as np

B, T, D = 4, 2048, 512
H, HD = 8, 64
Q = T // 2          # query rows per core
NCORES = 8
KT = T // 128       # 16 k-tiles
KP = KT // 2        # 8 k-pairs (DoubleRow)
QT = Q // 128       # 8 q-tiles
DC = D // 128       # 4 contraction chunks
SCALE = 1.0 / np.sqrt(HD)  # 0.125
EPS = 1e-5
MASK_BIAS = -1e9
LNP = float(np.log(64.0))      # P scaled by 1/64: max raw score is 71.9
                               # (9 sigma tails), exp(71.9/8)/64 = 125 < 240
SCHRAU_A = 8.0 / np.log(2.0)   # fp8e4 bits per e-fold
SCHRAU_B = 7.65                # (7-6)*8 (exp bias 7, scale 2^-6) - 0.35 centering
VP = 80                        # fp8 V row pitch (65 used, 16B-aligned)

# exp engine split per head: ACT is the exp bottleneck, DVE takes the tiles
# it has slack for (head 0 gets extra since its other DVE work moved to
# ACT; middle pairs so the attention@V consumer queue has drained).
# Overridden to "all ACT" by test.py --sim (CoreSim's u8 convert wraps
# instead of saturating).
DVE_EXP = {0: (6, 12), 1: (5, 10, 14), 2: (3, 7, 11, 14),
           3: (3, 7, 11, 14), 4: (2, 5, 8, 11, 14), 5: (2, 5, 8, 11, 13, 15),
           6: (2, 5, 8, 11, 13, 15), 7: (1, 3, 5, 7, 9, 11, 13)}

_CACHE = {}


def _emit(nc, tc, tens, affine):
    import contextlib

    import concourse.bass as bass
    from concourse import mybir
    from concourse.masks import make_identity

    f32 = mybir.dt.float32
    bf16 = mybir.dt.bfloat16
    f8 = mybir.dt.float8e4
    u8 = mybir.dt.uint8
    Alu = mybir.AluOpType
    Act = mybir.ActivationFunctionType
    DR = mybir.MatmulPerfMode.DoubleRow

    with contextlib.ExitStack() as stack:
        persist = stack.enter_context(tc.tile_pool(name="persist", bufs=1))
        small = stack.enter_context(tc.tile_pool(name="small", bufs=8))
        expp = stack.enter_context(tc.tile_pool(name="expp", bufs=KP + 4))
        otsbp = stack.enter_context(tc.tile_pool(name="otsbp", bufs=2))
        outp = stack.enter_context(tc.tile_pool(name="outp", bufs=5))
        pps = stack.enter_context(tc.tile_pool(name="pps", bufs=2, space="PSUM"))
        stp = stack.enter_context(tc.tile_pool(name="stp", bufs=2, space="PSUM"))
        scr = stack.enter_context(tc.tile_pool(name="scr", bufs=2, space="PSUM"))

        # ---- persistent tiles ----
        wt_sb = persist.tile([128, DC, 2 * D], bf16, name="wtsb", tag="wtsb")
        xt_sb = persist.tile([128, DC, T], bf16, name="xtsb", tag="xtsb")
        kt_sb = [persist.tile([128, T], bf16, name=f"ktsb{i}", tag=f"ktsb{i}")
                 for i in range(DC)]
        # Per-head Q^T padded to 128 contraction rows: rows (h%2)*64..+64 hold
        # Q_h, the other 64 rows stay zero. Keeps the score matmuls at K=128 —
        # K=64 matmuls don't register as PE activity for the HAM clock gate
        # and leave the whole attention phase throttled to 1.2 GHz.
        qt_pad = [persist.tile([128, Q], bf16, name=f"qtpad{h}", tag=f"qtpad{h}")
                  for h in range(H)]
        # X^T and W_v in fp8 d-pair layout for DoubleRow V-projection:
        # slice s of x8a holds X^T rows s*128..(s+1)*128
        x8p = [persist.tile([128, 2, T], f8, name=f"x8p{j}", tag=f"x8p{j}")
               for j in range(2)]
        wv8 = [persist.tile([128, 2, D], f8, name=f"wv8{j}", tag=f"wv8{j}")
               for j in range(2)]
        # V in fp8, k-pair major for DoubleRow: [k-part, pair-slice, head, col]
        # col 0 = 1.0 (denominator), cols 1:65 = V_h, 65:80 pad (16B stride).
        v8_sb = [persist.tile([128, 2, H, VP], f8, name=f"v8sb{p}",
                              tag=f"v8sb{p}") for p in range(KP)]
        oacc = [persist.tile([128, D], f32, name=f"oacc{q}", tag=f"oacc{q}")
                for q in range(QT)]
        rs_all = persist.tile([128, QT, H], f32, name="rs_all", tag="rs_all")
        ssq8 = persist.tile([128, QT], f32, name="ssq8", tag="ssq8")
        mean8 = persist.tile([128, QT], f32, name="mean8", tag="mean8")
        rstd8 = persist.tile([128, QT], f32, name="rstd8", tag="rstd8")
        xres_sb = persist.tile([128, QT, D], f32, name="xres_sb", tag="xres_sb")
        # smalls = [btr (12) | maska (16) | maskd (16)]
        smalls = persist.tile([128, 44], f32, name="smalls", tag="smalls")
        btr_sb = smalls[:, 0:12]
        maska_sb = smalls[:, 12:28]
        maskd_sb = smalls[:, 28:44]
        if affine:
            lnw_sb = persist.tile([128, D], f32, name="lnw_sb", tag="lnw_sb")
            lnb_sb = persist.tile([128, D], f32, name="lnb_sb", tag="lnb_sb")
        ident65 = persist.tile([HD + 1, HD + 1], bf16, name="ident65",
                               tag="ident65")
        wm_sb = persist.tile([128, 640], bf16, name="wm_sb", tag="wm_sb")

        # ---- input DMAs. Each issuing queue (sync/scalar/gpsimd) feeds its
        # own DMA ring at ~100GB/s, so the critical tensors are SPLIT across
        # queues to run the rings in parallel; issues stay batched (3D APs
        # over the host-pre-shaped [128, 4, cols] layouts) so per-issue
        # queue cost (~0.65us) stays small.
        # scalar ring: half of the first xt wave, then the queue turns to
        # compute (act-table dummy, block-0 copies, exp stream).
        nc.scalar.dma_start(out=xt_sb[:, 0:2, 0:512],
                            in_=tens["xt"][:, 0:2, 0:512])
        nc.scalar.dma_start(out=xt_sb[:, 0:2, 512:1024],
                            in_=tens["xt"][:, 0:2, 512:1024])
        # sync ring: critical wt prefix, the other xt halves, late params.
        nc.sync.dma_start(out=wt_sb[:, :, 0:256],
                          in_=tens["wt"][:, :, 0:256])
        nc.sync.dma_start(out=xt_sb[:, 2:4, 0:512],
                          in_=tens["xt"][:, 2:4, 0:512])
        nc.sync.dma_start(out=xt_sb[:, 2:4, 512:1024],
                          in_=tens["xt"][:, 2:4, 512:1024])
        nc.sync.dma_start(out=xt_sb[:, :, 1024:1536],
                          in_=tens["xt"][:, :, 1024:1536])
        nc.sync.dma_start(out=xt_sb[:, :, 1536:2048],
                          in_=tens["xt"][:, :, 1536:2048])
        nc.sync.dma_start(out=wt_sb[:, :, 256:1024],
                          in_=tens["wt"][:, :, 256:1024])
        nc.sync.dma_start(out=xres_sb, in_=tens["xres"][:])
        if affine:
            for dst, key in ((lnw_sb, "lnw"), (lnb_sb, "lnb")):
                src_ap = tens[key][:]
                ap = bass.AP(tensor=src_ap.tensor, offset=src_ap.offset,
                             ap=[[0, 128]] + list(src_ap.ap))
                nc.sync.dma_start(out=dst, in_=ap)
        nc.vector.memset(wm_sb, 0.5)
        # gpsimd ring: the small packed tile, V weights, then x8p in t-halves
        # (emit_v(k<8) needs only cols 0:1024 of both j slices), memsets
        # interleaved.
        nc.gpsimd.dma_start(out=smalls, in_=tens["smalls"][:])
        for h in range(2):
            z0 = 64 * (1 - (h % 2))
            nc.gpsimd.memset(qt_pad[h][z0:z0 + HD, :], 0.0)
        for j in range(2):
            nc.gpsimd.dma_start(out=wv8[j], in_=tens["wv8"][j])
        for j in range(2):
            nc.gpsimd.dma_start(out=x8p[j][:, :, 0:1024],
                                in_=tens["x8p"][j, :, :, 0:1024])
        for p in range(KP // 2):
            nc.gpsimd.memset(v8_sb[p][:, :, :, 0:1], 1.0)
        for j in range(2):
            nc.gpsimd.dma_start(out=x8p[j][:, :, 1024:2048],
                                in_=tens["x8p"][j, :, :, 1024:2048])
        for p in range(KP // 2, KP):
            nc.gpsimd.memset(v8_sb[p][:, :, :, 0:1], 1.0)
        for h in range(2, H):
            z0 = 64 * (1 - (h % 2))
            nc.gpsimd.memset(qt_pad[h][z0:z0 + HD, :], 0.0)
        make_identity(nc, ident65)

        # ---- scalar queue opener: a tiny dummy Exp so the ACT table load
        # executes during the initial DMA wait, not before the first real
        # exp tile.
        dummy8 = small.tile([128, 1], f8, name="dummy8", tag="dummy8")
        nc.scalar.activation(out=dummy8, in_=wm_sb[:, 0:1], func=Act.Exp)

        # ---- PE warm-up: K=128 matmuls with no data deps run during the
        # initial DMA wait so the HAM clock gate is already opening when
        # the projections start. The result is never used.
        wmps = stp.tile([128, Q], f32, name="wmps", tag="st")
        for i in range(6):
            nc.tensor.matmul(wmps[:, 0:512], wm_sb[:, 0:128],
                             wm_sb[:, 128:640], start=True, stop=True)
        wm_out = small.tile([128, 1], f32, name="wm_out", tag="wm_out")
        nc.vector.tensor_copy(out=wm_out, in_=wmps[:, 0:1])

        # ---- projection emitters. Block 0 copies ride the ACT engine
        # (idle until the exp stream starts); later blocks use DVE. ----
        def kt_chunk(i, tcn, on_act=False):
            ps = pps.tile([128, 512], f32, name="kps", tag="pps")
            for dc in range(DC):
                nc.tensor.matmul(
                    ps, wt_sb[:, dc, 256 * i: 256 * i + 128],
                    xt_sb[:, dc, tcn * 512:(tcn + 1) * 512],
                    start=(dc == 0), stop=(dc == DC - 1))
            dst = kt_sb[i][:, tcn * 512:(tcn + 1) * 512]
            if on_act:
                nc.scalar.activation(out=dst, in_=ps, func=Act.Identity,
                                     bias=btr_sb[:, 4 + i:5 + i])
            else:
                nc.vector.tensor_scalar_add(out=dst, in0=ps,
                                            scalar1=btr_sb[:, 4 + i:5 + i])

        def qt_chunk(i, qcn, on_act=False):
            ps = pps.tile([128, 512], f32, name="qps", tag="pps")
            for dc in range(DC):
                nc.tensor.matmul(
                    ps, wt_sb[:, dc, 256 * i + 128: 256 * i + 256],
                    xt_sb[:, dc, qcn * 512:(qcn + 1) * 512],
                    start=(dc == 0), stop=(dc == DC - 1))
            for j in range(2):
                r0 = j * HD
                dst = qt_pad[2 * i + j][r0:r0 + HD,
                                        qcn * 512:(qcn + 1) * 512]
                if on_act:
                    nc.scalar.activation(out=dst, in_=ps[r0:r0 + HD, :],
                                         func=Act.Identity,
                                         bias=btr_sb[r0:r0 + HD, i:i + 1])
                else:
                    nc.vector.tensor_scalar_add(
                        out=dst, in0=ps[r0:r0 + HD, :],
                        scalar1=btr_sb[r0:r0 + HD, i:i + 1])

        def emit_v(k, on_act=False):
            # fp8 DoubleRow projection (2 matmuls contract all 512 d-rows).
            # V-bias is folded into xres host-side (attn-out = sum P (v+bv)
            # / sum P = attn + bv), so the copy is a pure PSUM->fp8 convert.
            ps = pps.tile([128, 512], f32, name="vps", tag="pps")
            for j in range(2):
                nc.tensor.matmul(
                    ps, x8p[j][:, :, k * 128:(k + 1) * 128], wv8[j][:],
                    start=(j == 0), stop=(j == 1), perf_mode=DR)
            dst = v8_sb[k // 2][:, k % 2, :, 1:HD + 1]
            src = ps.rearrange("p (h d) -> p h d", h=H)
            if on_act:
                nc.scalar.activation(out=dst, in_=src, func=Act.Copy)
            else:
                nc.vector.tensor_copy(out=dst, in_=src)

        # ---- LayerNorm group: stats + normalize (+ optional affine) for
        # the 2 q-tiles of one last-head chunk. var = E[y^2] - mean^2. ----
        def emit_ln_group(qg):
            g = slice(qg * 2, qg * 2 + 2)
            rowsum2 = small.tile([128, 2], f32, name="rowsum2",
                                 tag="rowsum2")
            nc.vector.reduce_sum(out=rowsum2, in_=rs_all[:, g, :],
                                 axis=mybir.AxisListType.X)
            nc.vector.tensor_scalar_mul(out=mean8[:, g], in0=rowsum2,
                                        scalar1=1.0 / D)
            msq = small.tile([128, 2], f32, name="msq", tag="msq")
            nc.vector.tensor_tensor(out=msq, in0=mean8[:, g],
                                    in1=mean8[:, g], op=Alu.mult)
            var2 = small.tile([128, 2], f32, name="var2", tag="var2")
            nc.vector.tensor_scalar(out=var2, in0=ssq8[:, g],
                                    scalar1=1.0 / D, scalar2=EPS,
                                    op0=Alu.mult, op1=Alu.add)
            varc = small.tile([128, 2], f32, name="varc", tag="varc")
            nc.vector.tensor_tensor(out=varc, in0=var2, in1=msq,
                                    op=Alu.subtract)
            sd2 = small.tile([128, 2], f32, name="sd2", tag="sd2")
            nc.scalar.activation(out=sd2, in_=varc, func=Act.Sqrt)
            nc.vector.reciprocal(out=rstd8[:, g], in_=sd2)
            for q in range(qg * 2, qg * 2 + 2):
                yn = outp.tile([128, D], f32, name="yn", tag="yn")
                nc.vector.tensor_scalar(
                    out=yn, in0=oacc[q], scalar1=mean8[:, q:q + 1],
                    scalar2=rstd8[:, q:q + 1],
                    op0=Alu.subtract, op1=Alu.mult)
                if affine:
                    eng = nc.gpsimd if q % 2 else nc.vector
                    yw = outp.tile([128, D], f32, name="yw", tag="yw")
                    eng.tensor_tensor(out=yw, in0=yn, in1=lnw_sb, op=Alu.mult)
                    yo = outp.tile([128, D], f32, name="yo", tag="yo")
                    eng.tensor_tensor(out=yo, in0=yw, in1=lnb_sb, op=Alu.add)
                else:
                    yo = yn
                nc.sync.dma_start(out=tens["out"][q * 128:(q + 1) * 128, :],
                                  in_=yo)

        # ---- attention head emitters ----
        head_pairs = {}

        def epilogue_q(h, otsb_tile, col0, q):
            tp = pps.tile([128, HD + 1], bf16, name="tp", tag="pps")
            nc.tensor.transpose(tp, otsb_tile[:, col0:col0 + 128], ident65)
            rec = small.tile([128, 1], f32, name="rec", tag="rec")
            nc.vector.reciprocal(out=rec, in_=tp[:, 0:1])
            nc.vector.scalar_tensor_tensor(
                out=oacc[q][:, h * HD:(h + 1) * HD],
                in0=tp[:, 1:HD + 1], scalar=rec, op0=Alu.mult,
                in1=xres_sb[:, q, h * HD:(h + 1) * HD], op1=Alu.add,
                accum_out=rs_all[:, q, h:h + 1])
            if h == H - 1:
                # sum of squares for LayerNorm variance (E[y^2] - mean^2).
                # Early chunks square on DVE (y*y via stt + accum) so the
                # final chunk's ACT chain (its 2 squares + sqrt) is short —
                # ACT's in-order queue would otherwise make the last chunk
                # wait behind all earlier squares.
                sqs = outp.tile([128, D], f32, name="sqs", tag="sqs")
                if q < 6:
                    nc.vector.scalar_tensor_tensor(
                        out=sqs, in0=oacc[q], scalar=1.0, op0=Alu.mult,
                        in1=oacc[q], op1=Alu.mult,
                        accum_out=ssq8[:, q:q + 1])
                else:
                    nc.scalar.activation(out=sqs, in_=oacc[q],
                                         func=Act.Square,
                                         accum_out=ssq8[:, q:q + 1])

        def emit_score_tile(h, k, pairs):
            """Scores for one k-tile + engine-split exp into pair tile."""
            blk = h // 2
            st = stp.tile([128, Q], f32, name="st", tag="st")
            for qcn in range(Q // 512):
                nc.tensor.matmul(
                    st[:, qcn * 512:(qcn + 1) * 512],
                    kt_sb[blk][:, k * 128:(k + 1) * 128],
                    qt_pad[h][:, qcn * 512:(qcn + 1) * 512],
                    start=None, stop=None)
            if k % 2 == 0:
                pairs.append(expp.tile([128, 2, Q], f8, name="ppair",
                                       tag="ppair"))
            pt = pairs[k // 2]
            if k not in DVE_EXP[h]:
                nc.scalar.activation(out=pt[:, k % 2, :], in_=st,
                                     func=Act.Exp,
                                     bias=maska_sb[:, k:k + 1], scale=SCALE)
            else:
                nc.vector.tensor_scalar(
                    out=pt[:, k % 2, :].bitcast(u8), in0=st,
                    scalar1=float(SCALE * SCHRAU_A),
                    scalar2=maskd_sb[:, k:k + 1],
                    op0=Alu.mult, op1=Alu.add)

        def emit_scores(h):
            pairs = head_pairs[h] = []
            for k in range(KT):
                emit_score_tile(h, k, pairs)

        def av_pair(h, ots, kp, qcn_range=(0, 1)):
            pairs = head_pairs[h]
            for qcn in qcn_range:
                nc.tensor.matmul(
                    ots[qcn], v8_sb[kp][:, :, h, 0:HD + 1],
                    pairs[kp][:, :, qcn * 512:(qcn + 1) * 512],
                    start=(kp == 0), stop=(kp == KP - 1),
                    perf_mode=DR)

        def emit_av(h, inter_with=None, ots=None, done_pairs=0, extra=()):
            pairs = head_pairs[h]
            extra = list(extra)
            # O^T[1+d, q] accumulated over k-pairs via fp8 DoubleRow; V_h
            # stationary so its weight load hides behind the 512-col moving
            # stream. Interleaved per k-pair with the NEXT head's score/exp
            # emission (and any deferred projection chunks) so ACT/PE never
            # starve behind a dense attention@V block.
            if ots is None:
                ots = [scr.tile([HD + 1, 512], f32, name=f"ot{qcn}", tag="ot")
                       for qcn in range(Q // 512)]
            if h != H - 1:
                otsb = [otsbp.tile([HD + 1, 512], bf16, name=f"otsb{qcn}",
                                   tag=f"otsb{qcn}") for qcn in range(Q // 512)]
                if inter_with is not None:
                    npairs = head_pairs[inter_with] = []
                for kp in range(done_pairs, KP):
                    av_pair(h, ots, kp)
                    if inter_with is not None:
                        emit_score_tile(inter_with, 2 * kp, npairs)
                        emit_score_tile(inter_with, 2 * kp + 1, npairs)
                    if extra:
                        extra.pop(0)()
                for qcn in range(Q // 512):
                    nc.vector.tensor_copy(out=otsb[qcn], in_=ots[qcn])
                for q in range(QT):
                    epilogue_q(h, otsb[q // 4], (q % 4) * 128, q)
            else:
                # last head: 4 chunks of 256 query columns; each chunk's 2
                # q-tiles run their epilogue + LayerNorm group immediately,
                # so only the final chunk's epilogue trails the last matmul
                for qg in range(4):
                    qcn, c0 = qg // 2, (qg % 2) * 256
                    for kp in range(KP):
                        nc.tensor.matmul(
                            ots[qcn][:, c0:c0 + 256],
                            v8_sb[kp][:, :, h, 0:HD + 1],
                            pairs[kp][:, :, qg * 256:(qg + 1) * 256],
                            start=(kp == 0), stop=(kp == KP - 1),
                            perf_mode=DR)
                    otsb = otsbp.tile([HD + 1, 256], bf16, name="otsbc",
                                      tag="otsbc")
                    nc.vector.tensor_copy(out=otsb,
                                          in_=ots[qcn][:, c0:c0 + 256])
                    for qi in range(2):
                        epilogue_q(h, otsb, qi * 128, qg * 2 + qi)
                    emit_ln_group(qg)

        # ---- emission. Block-0 projections interleave with head 0's first
        # score tiles so the exp stream starts as early as the DMA critical
        # path allows; V-proj and attention@V fill the PE queue behind it.
        kt_chunk(0, 0, on_act=True)
        qt_chunk(0, 0, on_act=True)
        qt_chunk(0, 1, on_act=True)
        kt_chunk(0, 1, on_act=True)
        pairs0 = head_pairs[0] = []
        ots0 = [scr.tile([HD + 1, 512], f32, name=f"ot{qcn}", tag="ot")
                for qcn in range(Q // 512)]
        for kp in range(KP):
            emit_score_tile(0, 2 * kp, pairs0)
            emit_score_tile(0, 2 * kp + 1, pairs0)
            if kp == 1:
                kt_chunk(0, 2, on_act=True)
            if kp == 2:
                kt_chunk(0, 3, on_act=True)
            emit_v(2 * kp, on_act=(kp < 4))
            emit_v(2 * kp + 1, on_act=(kp < 4))
            av_pair(0, ots0, kp)
        # head 1 scores standalone (ACT-bound stretch: deferred block-1
        # projections slot between score tiles without starving exp)
        from functools import partial
        blk1 = [partial(kt_chunk, 1, t) for t in range(4)] + [
            partial(qt_chunk, 1, c) for c in range(2)]
        pairs1 = head_pairs[1] = []
        for k in range(KT):
            emit_score_tile(1, k, pairs1)
            if k % 3 == 2 and blk1:
                blk1.pop(0)()
        while blk1:
            blk1.pop(0)()
        emit_av(0, ots=ots0, done_pairs=KP)
        blk2 = [partial(kt_chunk, 2, t) for t in range(4)] + [
            partial(qt_chunk, 2, c) for c in range(2)]
        emit_av(1, inter_with=2, extra=blk2[:3])
        emit_av(2, inter_with=3, extra=blk2[3:])
        blk3 = [partial(kt_chunk, 3, t) for t in range(4)] + [
            partial(qt_chunk, 3, c) for c in range(2)]
        emit_av(3, inter_with=4, extra=blk3[:3])
        emit_av(4, inter_with=5, extra=blk3[3:])
        emit_av(5, inter_with=6)
        emit_av(6, inter_with=7)
        emit_av(H - 1)

        # (residual + LayerNorm is emitted per chunk from the last head)


def _build(affine):
    import concourse.bacc as bacc
    import concourse.tile as tile
    from concourse import mybir

    f32 = mybir.dt.float32
    bf16 = mybir.dt.bfloat16
    nc = bacc.Bacc("TRN2", target_bir_lowering=False, debug=False)

    tens = {
        "xt": nc.dram_tensor("xt", [128, DC, T], bf16, kind="ExternalInput"),
        "xres": nc.dram_tensor("xres", [128, QT, D], f32,
                               kind="ExternalInput"),
        "wt": nc.dram_tensor("wt", [128, DC, 2 * D], bf16,
                             kind="ExternalInput"),
        "x8p": nc.dram_tensor("x8p", [2, 128, 2, T], mybir.dt.float8e4,
                              kind="ExternalInput"),
        "wv8": nc.dram_tensor("wv8", [2, 128, 2, D], mybir.dt.float8e4,
                              kind="ExternalInput"),
        "smalls": nc.dram_tensor("smalls", [128, 44], f32,
                                 kind="ExternalInput"),
        "out": nc.dram_tensor("out", [Q, D], f32, kind="ExternalOutput"),
    }
    if affine:
        tens["lnw"] = nc.dram_tensor("lnw", [D], f32, kind="ExternalInput")
        tens["lnb"] = nc.dram_tensor("lnb", [D], f32, kind="ExternalInput")

    with tile.TileContext(nc) as tc:
        _emit(nc, tc, tens, affine)
    nc.compile()
    return nc


def make_in_maps(query, key_mask, in_proj_weight, in_proj_bias, ln_weight,
                 ln_bias):
    import ml_dtypes

    bf = ml_dtypes.bfloat16
    query = np.asarray(query, dtype=np.float32)
    key_mask = np.asarray(key_mask)
    w = np.asarray(in_proj_weight, dtype=np.float32)
    b = np.asarray(in_proj_bias, dtype=np.float32)
    lnw = np.asarray(ln_weight, dtype=np.float32)
    lnb = np.asarray(ln_bias, dtype=np.float32)
    affine = not (np.all(lnw == 1.0) and np.all(lnb == 0.0))

    # wt host layout: [K_blk0 | Q_blk0 | K_blk1 | Q_blk1 | ...] 128-col
    # groups, rows regrouped [128, 4, cols] so the critical projection
    # weights arrive in one DMA issue. V columns live only in wv8.
    wcols = []
    for i in range(DC):
        wcols.append(w.T[:, D + 128 * i: D + 128 * (i + 1)])  # K block i
        wcols.append(w.T[:, 128 * i: 128 * (i + 1)])          # Q block i
    wtr = np.concatenate(wcols, axis=1).astype(bf)            # [512, 1024]
    wt = np.ascontiguousarray(wtr.reshape(DC, 128, 2 * D).transpose(1, 0, 2))
    btr = np.ascontiguousarray(b.reshape(12, 128).T)
    bv = b[2 * D:3 * D]  # folded into xres: attn-out(v+bv) = attn-out(v)+bv
    in_maps = []
    for c in range(NCORES):
        bi, half = c // 2, c % 2
        xb = query[bi]
        # k-columns reordered so this core's query half sits at 0:Q — the
        # Q-projection then reads xt directly (no separate xq input) and
        # attention is permutation-invariant over k as long as the mask
        # follows the same order.
        perm = (np.r_[Q:T, 0:Q] if half else np.arange(T))
        xbt = np.ascontiguousarray(
            xb.T[:, perm].astype(bf).reshape(DC, 128, T).transpose(1, 0, 2))
        # fp8 operands quantized straight from f32: rounding f32->bf16->fp8
        # instead costs 1.5x in final max-error (boundary double rounding)
        f8 = ml_dtypes.float8_e4m3
        xbt8 = xb.T[:, perm].astype(f8)
        x8p = np.ascontiguousarray(
            xbt8.reshape(2, 2, 128, T).transpose(0, 2, 1, 3))
        wv8 = np.ascontiguousarray(
            w[2 * D:3 * D].T.astype(f8)
            .reshape(2, 2, 128, D).transpose(0, 2, 1, 3))
        km = key_mask[bi][perm]
        maskb = np.where(km, np.float32(MASK_BIAS), np.float32(0.0))
        maska = (maskb - LNP).astype(np.float32).reshape(KT, 128).T
        maskd = np.where(km, np.float32(-1e6),
                         np.float32(SCHRAU_B)).reshape(KT, 128).T
        smalls = np.concatenate([btr, maska, maskd], axis=1).astype(np.float32)
        xres = (xb[half * Q:(half + 1) * Q] + bv[None, :]).astype(np.float32)
        im = {
            "xt": xbt,
            "xres": np.ascontiguousarray(
                xres.reshape(QT, 128, D).transpose(1, 0, 2)),
            "wt": wt,
            "x8p": x8p,
            "wv8": wv8,
            "smalls": np.ascontiguousarray(smalls),
        }
        if affine:
            im["lnw"] = lnw
            im["lnb"] = lnb
        in_maps.append(im)
    return in_maps


def assemble(results):
    out = np.empty((B, T, D), dtype=np.float32)
    for c in range(NCORES):
        bi, half = c // 2, c % 2
        out[bi, half * Q:(half + 1) * Q] = results[c]["out"]
    return out


def get_nc(affine=False):
    key = ("nc", affine)
    if key not in _CACHE:
        _CACHE[key] = _build(affine)
    return _CACHE[key]


def kernel(query, key_mask, in_proj_weight, in_proj_bias, ln_weight, ln_bias):
    from concourse.bass_utils import run_bass_kernel_spmd

    affine = not (np.all(np.asarray(ln_weight) == 1.0)
                  and np.all(np.asarray(ln_bias) == 0.0))
    nc = get_nc(affine)
    in_maps = make_in_maps(query, key_mask, in_proj_weight, in_proj_bias,
                           ln_weight, ln_bias)
    res = run_bass_kernel_spmd(nc, in_maps, core_ids=list(range(NCORES)))
    return assemble(res.results)


# revision 13
# speedup vs baseline: 1.1942x; 1.1942x over previous
"""Trainium2 Bass kernel for fused MultiHeadAttention + residual + LayerNorm.

Problem: query [4, 2048, 512] f32, H=8 heads (hd=64), fused QKV projection,
key-padding-mask softmax, attn @ V, residual add, LayerNorm over D=512.

Sharding: 8 cores = 4 batches x 2 query-halves. Each core handles one batch's
full K/V (T=2048) and 1024 query rows, so heads stay local and the output
LayerNorm needs no cross-core communication. K/V projection is duplicated
between the 2 cores sharing a batch; X^T columns are rotated per core so its
own query half sits at columns 0:Q (Q^T projects straight out of X^T, no
separate xq input, and attention is k-permutation-invariant with the mask
rotated to match).

K/Q projections and scores run in bf16 (fp32 PSUM accum) — fp8 K-proj was
measured offline at 2.6e-2 max error (score errors amplify through exp),
past the 2e-2 budget, so K/Q must stay bf16; score matmuls are 2x512-col
(a single 1024-col moving matmul fails the ISA check). The V projection,
attention weights P = exp(S/8 - ln64) and V run fp8e4 so those matmuls use
DoubleRow perf mode: one pass contracts TWO 128-row slices at the same
1 col/cycle -> half the PE time. fp8 operands are quantized on the host
straight from f32 (f32->bf16->fp8 double rounding costs 1.5x in max error).
The 1/64 P scaling keeps exp() inside e4m3 range for the 9-sigma score
tails (max raw score 71.9); the denominator (ones-column 0 of each V head
group) scales identically so the softmax ratio is unaffected.

Softmax exp is the Scalar/ACT bottleneck (128 tiles x [128,1024], ~1.07us
each on ACT / ~1.28us on DVE per the measured trace), so per head several
tiles go to DVE via the Schraudolph bit-trick
  fp8bits(exp(s)) ~= uint8(s*(8/ln2)*SCALE + 7.65 + maskbias)
(f32->uint8 convert is round-to-nearest saturating to [0,255] on HW, so the
exp underflow tail AND masked rows — addend -1e6 — clamp to +0.0; scores
never reach the bits>=120 inf/nan region). The uint8 tile is bitcast to
fp8e4 for the DoubleRow matmul. ~4% rms error on those P tiles, attenuated
~25x by the softmax-weighted average + f32 residual, keeps the total error
under the 2e-2 budget. GpSimd cannot read PSUM so it takes SBUF-only
work (memsets, identity).

Per-core flow:
  X^T [128,4,2048] bf16, W^T [128,4,1024] bf16 (host-interleaved
  [K_i|Q_i] 128-col groups; V columns live only in the fp8 operands),
  K^T [512,2048] bf16, Q^T head-major zero-padded to K=128 contraction
  V8  8 pair-tiles [128,2,H,80] fp8  (col 0 of each head group = 1.0)
  S^T [128k,1024q] f32 PSUM per (head,k-tile) -> exp (ACT fp8 out / DVE
      u8 trick) -> P pair tiles [128,2,1024] fp8
  O^T [65,1024] f32 = [1|V_h].T @dr P^T accumulated over 8 k-pairs
  bf16 copy -> PE-transpose [65,128]->[128,65], DVE reciprocal(denom),
  fused multiply-add folds the residual in per head slice; the recombine's
  accum_out collects row-sums for the LayerNorm mean
  LayerNorm: var = E[y^2]-mean^2 with ACT Square+accum, batched Sqrt, DVE
  normalize -> DMA out f32. When ln_weight==1 and ln_bias==0 (runtime
  check, true for this problem) the gamma/beta affine stage is skipped
  entirely; otherwise a second kernel variant with the affine ops compiles.

Scheduling (v2, driven by the measured per-instruction trace):
  - ~6us multicore-barrier preamble is fixed; the critical input DMAs are
    batched into single 3D issues (host pre-shapes xt/wt as [128,4,cols])
    so the first K-proj starts ~10us: sync queue carries xt in 512-col
    waves then xres/ln params; vector carries wt (critical 256-col prefix
    first) then the fp8 V operands; gpsimd carries the packed
    btr|maska|maskd tile then memsets; scalar carries NO DMA — it opens
    with a tiny dummy Exp (hoists the ACT table load into the DMA wait)
    then does the block-0 K/Q PSUM->SBUF copies (Identity+bias) while the
    exp stream hasn't started, then V-proj casts (alternating with DVE).
  - block-0 emission interleaves kt chunks / qt chunks / head-0 score
    tiles so the first exp fires right after the first two score matmuls.
  - every later attention@V interleaves per k-pair with the next head's
    score tiles and deferred projection chunks (block b complete before
    av(2b-1)).
  - the last head runs attention@V in 4 chunks of 256 query columns;
    after each chunk its 2 q-tiles run transpose/recombine/Square and a
    per-2-tile LayerNorm group, so only ~1/4 of the epilogue remains
    after the final matmul (the v1 two-group epilogue left a ~28us
    serial tail).
"""

import numpy as np

B, T, D = 4, 2048, 512
H, HD = 8, 64
Q = T // 2          # query rows per core
NCORES = 8
KT = T // 128       # 16 k-tiles
KP = KT // 2        # 8 k-pairs (DoubleRow)
QT = Q // 128       # 8 q-tiles
DC = D // 128       # 4 contraction chunks
SCALE = 1.0 / np.sqrt(HD)  # 0.125
EPS = 1e-5
MASK_BIAS = -1e9
LNP = float(np.log(64.0))      # P scaled by 1/64: max raw score is 71.9
                               # (9 sigma tails), exp(71.9/8)/64 = 125 < 240
SCHRAU_A = 8.0 / np.log(2.0)   # fp8e4 bits per e-fold
SCHRAU_B = 7.65                # (7-6)*8 (exp bias 7, scale 2^-6) - 0.35 centering
VP = 80                        # fp8 V row pitch (65 used, 16B-aligned)

# exp engine split per head: ACT is the exp bottleneck, DVE takes the tiles
# it has slack for (head 0 gets extra since its other DVE work moved to
# ACT; middle pairs so the attention@V consumer queue has drained).
# Overridden to "all ACT" by test.py --sim (CoreSim's u8 convert wraps
# instead of saturating).
DVE_EXP = {0: (6, 12), 1: (5, 10, 14), 2: (3, 7, 11, 14),
           3: (3, 7, 11, 14), 4: (2, 5, 8, 11, 14), 5: (2, 5, 8, 11, 13, 15),
           6: (2, 5, 8, 11, 13, 15), 7: (1, 3, 5, 7, 9, 11, 13)}

_CACHE = {}


def _emit(nc, tc, tens, affine):
    import contextlib

    import concourse.bass as bass
    from concourse import mybir
    from concourse.masks import make_identity

    f32 = mybir.dt.float32
    bf16 = mybir.dt.bfloat16
    f8 = mybir.dt.float8e4
    u8 = mybir.dt.uint8
    Alu = mybir.AluOpType
    Act = mybir.ActivationFunctionType
    DR = mybir.MatmulPerfMode.DoubleRow

    with contextlib.ExitStack() as stack:
        persist = stack.enter_context(tc.tile_pool(name="persist", bufs=1))
        small = stack.enter_context(tc.tile_pool(name="small", bufs=8))
        expp = stack.enter_context(tc.tile_pool(name="expp", bufs=KP + 4))
        otsbp = stack.enter_context(tc.tile_pool(name="otsbp", bufs=2))
        outp = stack.enter_context(tc.tile_pool(name="outp", bufs=5))
        pps = stack.enter_context(tc.tile_pool(name="pps", bufs=2, space="PSUM"))
        stp = stack.enter_context(tc.tile_pool(name="stp", bufs=2, space="PSUM"))
        scr = stack.enter_context(tc.tile_pool(name="scr", bufs=2, space="PSUM"))

        # ---- persistent tiles ----
        wt_sb = persist.tile([128, DC, 2 * D], bf16, name="wtsb", tag="wtsb")
        xt_sb = persist.tile([128, DC, T], bf16, name="xtsb", tag="xtsb")
        kt_sb = [persist.tile([128, T], bf16, name=f"ktsb{i}", tag=f"ktsb{i}")
                 for i in range(DC)]
        # Per-head Q^T padded to 128 contraction rows: rows (h%2)*64..+64 hold
        # Q_h, the other 64 rows stay zero. Keeps the score matmuls at K=128 —
        # K=64 matmuls don't register as PE activity for the HAM clock gate
        # and leave the whole attention phase throttled to 1.2 GHz.
        qt_pad = [persist.tile([128, Q], bf16, name=f"qtpad{h}", tag=f"qtpad{h}")
                  for h in range(H)]
        # X^T and W_v in fp8 d-pair layout for DoubleRow V-projection:
        # slice s of x8a holds X^T rows s*128..(s+1)*128
        x8p = [persist.tile([128, 2, T], f8, name=f"x8p{j}", tag=f"x8p{j}")
               for j in range(2)]
        wv8 = [persist.tile([128, 2, D], f8, name=f"wv8{j}", tag=f"wv8{j}")
               for j in range(2)]
        # V in fp8, k-pair major for DoubleRow: [k-part, pair-slice, head, col]
        # col 0 = 1.0 (denominator), cols 1:65 = V_h, 65:80 pad (16B stride).
        v8_sb = [persist.tile([128, 2, H, VP], f8, name=f"v8sb{p}",
                              tag=f"v8sb{p}") for p in range(KP)]
        oacc = [persist.tile([128, D], f32, name=f"oacc{q}", tag=f"oacc{q}")
                for q in range(QT)]
        rs_all = persist.tile([128, QT, H], f32, name="rs_all", tag="rs_all")
        ssq8 = persist.tile([128, QT], f32, name="ssq8", tag="ssq8")
        mean8 = persist.tile([128, QT], f32, name="mean8", tag="mean8")
        rstd8 = persist.tile([128, QT], f32, name="rstd8", tag="rstd8")
        xres_sb = persist.tile([128, QT, D], f32, name="xres_sb", tag="xres_sb")
        # smalls = [btr (12) | maska (16) | maskd (16)]
        smalls = persist.tile([128, 44], f32, name="smalls", tag="smalls")
        btr_sb = smalls[:, 0:12]
        maska_sb = smalls[:, 12:28]
        maskd_sb = smalls[:, 28:44]
        if affine:
            lnw_sb = persist.tile([128, D], f32, name="lnw_sb", tag="lnw_sb")
            lnb_sb = persist.tile([128, D], f32, name="lnb_sb", tag="lnb_sb")
        ident65 = persist.tile([HD + 1, HD + 1], bf16, name="ident65",
                               tag="ident65")
        wm_sb = persist.tile([128, 640], bf16, name="wm_sb", tag="wm_sb")

        # ---- input DMAs. Each issuing queue (sync/scalar/gpsimd) feeds its
        # own DMA ring at ~100GB/s, so the critical tensors are SPLIT across
        # queues to run the rings in parallel; issues stay batched (3D APs
        # over the host-pre-shaped [128, 4, cols] layouts) so per-issue
        # queue cost (~0.65us) stays small.
        # scalar ring: half of the first xt wave, then the queue turns to
        # compute (act-table dummy, block-0 copies, exp stream).
        nc.scalar.dma_start(out=xt_sb[:, 0:2, 0:512],
                            in_=tens["xt"][:, 0:2, 0:512])
        nc.scalar.dma_start(out=xt_sb[:, 0:2, 512:1024],
                            in_=tens["xt"][:, 0:2, 512:1024])
        # sync ring: critical wt prefix, the other xt halves, late params.
        nc.sync.dma_start(out=wt_sb[:, :, 0:256],
                          in_=tens["wt"][:, :, 0:256])
        nc.sync.dma_start(out=xt_sb[:, 2:4, 0:512],
                          in_=tens["xt"][:, 2:4, 0:512])
        nc.sync.dma_start(out=xt_sb[:, 2:4, 512:1024],
                          in_=tens["xt"][:, 2:4, 512:1024])
        nc.sync.dma_start(out=xt_sb[:, :, 1024:1536],
                          in_=tens["xt"][:, :, 1024:1536])
        nc.sync.dma_start(out=xt_sb[:, :, 1536:2048],
                          in_=tens["xt"][:, :, 1536:2048])
        nc.sync.dma_start(out=wt_sb[:, :, 256:1024],
                          in_=tens["wt"][:, :, 256:1024])
        nc.sync.dma_start(out=xres_sb, in_=tens["xres"][:])
        if affine:
            for dst, key in ((lnw_sb, "lnw"), (lnb_sb, "lnb")):
                src_ap = tens[key][:]
                ap = bass.AP(tensor=src_ap.tensor, offset=src_ap.offset,
                             ap=[[0, 128]] + list(src_ap.ap))
                nc.sync.dma_start(out=dst, in_=ap)
        nc.vector.memset(wm_sb, 0.5)
        # gpsimd ring: the small packed tile, V weights, then x8p in t-halves
        # (emit_v(k<8) needs only cols 0:1024 of both j slices), memsets
        # interleaved.
        nc.gpsimd.dma_start(out=smalls, in_=tens["smalls"][:])
        for h in range(2):
            z0 = 64 * (1 - (h % 2))
            nc.gpsimd.memset(qt_pad[h][z0:z0 + HD, :], 0.0)
        for j in range(2):
            nc.gpsimd.dma_start(out=wv8[j], in_=tens["wv8"][j])
        for j in range(2):
            nc.gpsimd.dma_start(out=x8p[j][:, :, 0:1024],
                                in_=tens["x8p"][j, :, :, 0:1024])
        for p in range(KP // 2):
            nc.gpsimd.memset(v8_sb[p][:, :, :, 0:1], 1.0)
        for j in range(2):
            nc.gpsimd.dma_start(out=x8p[j][:, :, 1024:2048],
                                in_=tens["x8p"][j, :, :, 1024:2048])
        for p in range(KP // 2, KP):
            nc.gpsimd.memset(v8_sb[p][:, :, :, 0:1], 1.0)
        for h in range(2, H):
            z0 = 64 * (1 - (h % 2))
            nc.gpsimd.memset(qt_pad[h][z0:z0 + HD, :], 0.0)
        make_identity(nc, ident65)

        # ---- scalar queue opener: a tiny dummy Exp so the ACT table load
        # executes during the initial DMA wait, not before the first real
        # exp tile.
        dummy8 = small.tile([128, 1], f8, name="dummy8", tag="dummy8")
        nc.scalar.activation(out=dummy8, in_=wm_sb[:, 0:1], func=Act.Exp)

        # ---- PE warm-up: K=128 matmuls with no data deps run during the
        # initial DMA wait so the HAM clock gate is already opening when
        # the projections start. The result is never used.
        wmps = stp.tile([128, Q], f32, name="wmps", tag="st")
        for i in range(6):
            nc.tensor.matmul(wmps[:, 0:512], wm_sb[:, 0:128],
                             wm_sb[:, 128:640], start=True, stop=True)
        wm_out = small.tile([128, 1], f32, name="wm_out", tag="wm_out")
        nc.vector.tensor_copy(out=wm_out, in_=wmps[:, 0:1])

        # ---- projection emitters. Block 0 copies ride the ACT engine
        # (idle until the exp stream starts); later blocks use DVE. ----
        def kt_chunk(i, tcn, on_act=False):
            ps = pps.tile([128, 512], f32, name="kps", tag="pps")
            for dc in range(DC):
                nc.tensor.matmul(
                    ps, wt_sb[:, dc, 256 * i: 256 * i + 128],
                    xt_sb[:, dc, tcn * 512:(tcn + 1) * 512],
                    start=(dc == 0), stop=(dc == DC - 1))
            dst = kt_sb[i][:, tcn * 512:(tcn + 1) * 512]
            if on_act:
                nc.scalar.activation(out=dst, in_=ps, func=Act.Identity,
                                     bias=btr_sb[:, 4 + i:5 + i])
            else:
                nc.vector.tensor_scalar_add(out=dst, in0=ps,
                                            scalar1=btr_sb[:, 4 + i:5 + i])

        def qt_chunk(i, qcn, on_act=False):
            ps = pps.tile([128, 512], f32, name="qps", tag="pps")
            for dc in range(DC):
                nc.tensor.matmul(
                    ps, wt_sb[:, dc, 256 * i + 128: 256 * i + 256],
                    xt_sb[:, dc, qcn * 512:(qcn + 1) * 512],
                    start=(dc == 0), stop=(dc == DC - 1))
            for j in range(2):
                r0 = j * HD
                dst = qt_pad[2 * i + j][r0:r0 + HD,
                                        qcn * 512:(qcn + 1) * 512]
                if on_act:
                    nc.scalar.activation(out=dst, in_=ps[r0:r0 + HD, :],
                                         func=Act.Identity,
                                         bias=btr_sb[r0:r0 + HD, i:i + 1])
                else:
                    nc.vector.tensor_scalar_add(
                        out=dst, in0=ps[r0:r0 + HD, :],
                        scalar1=btr_sb[r0:r0 + HD, i:i + 1])

        def emit_v(k, on_act=False):
            # fp8 DoubleRow projection (2 matmuls contract all 512 d-rows).
            # V-bias is folded into xres host-side (attn-out = sum P (v+bv)
            # / sum P = attn + bv), so the copy is a pure PSUM->fp8 convert.
            ps = pps.tile([128, 512], f32, name="vps", tag="pps")
            for j in range(2):
                nc.tensor.matmul(
                    ps, x8p[j][:, :, k * 128:(k + 1) * 128], wv8[j][:],
                    start=(j == 0), stop=(j == 1), perf_mode=DR)
            dst = v8_sb[k // 2][:, k % 2, :, 1:HD + 1]
            src = ps.rearrange("p (h d) -> p h d", h=H)
            if on_act:
                nc.scalar.activation(out=dst, in_=src, func=Act.Copy)
            else:
                nc.vector.tensor_copy(out=dst, in_=src)

        # ---- LayerNorm group: stats + normalize (+ optional affine) for
        # the 2 q-tiles of one last-head chunk. var = E[y^2] - mean^2. ----
        def emit_ln_group(qg):
            g = slice(qg * 2, qg * 2 + 2)
            rowsum2 = small.tile([128, 2], f32, name="rowsum2",
                                 tag="rowsum2")
            nc.vector.reduce_sum(out=rowsum2, in_=rs_all[:, g, :],
                                 axis=mybir.AxisListType.X)
            nc.vector.tensor_scalar_mul(out=mean8[:, g], in0=rowsum2,
                                        scalar1=1.0 / D)
            msq = small.tile([128, 2], f32, name="msq", tag="msq")
            nc.vector.tensor_tensor(out=msq, in0=mean8[:, g],
                                    in1=mean8[:, g], op=Alu.mult)
            var2 = small.tile([128, 2], f32, name="var2", tag="var2")
            nc.vector.tensor_scalar(out=var2, in0=ssq8[:, g],
                                    scalar1=1.0 / D, scalar2=EPS,
                                    op0=Alu.mult, op1=Alu.add)
            varc = small.tile([128, 2], f32, name="varc", tag="varc")
            nc.vector.tensor_tensor(out=varc, in0=var2, in1=msq,
                                    op=Alu.subtract)
            sd2 = small.tile([128, 2], f32, name="sd2", tag="sd2")
            nc.scalar.activation(out=sd2, in_=varc, func=Act.Sqrt)
            nc.vector.reciprocal(out=rstd8[:, g], in_=sd2)
            for q in range(qg * 2, qg * 2 + 2):
                yn = outp.tile([128, D], f32, name="yn", tag="yn")
                nc.vector.tensor_scalar(
                    out=yn, in0=oacc[q], scalar1=mean8[:, q:q + 1],
                    scalar2=rstd8[:, q:q + 1],
                    op0=Alu.subtract, op1=Alu.mult)
                if affine:
                    eng = nc.gpsimd if q % 2 else nc.vector
                    yw = outp.tile([128, D], f32, name="yw", tag="yw")
                    eng.tensor_tensor(out=yw, in0=yn, in1=lnw_sb, op=Alu.mult)
                    yo = outp.tile([128, D], f32, name="yo", tag="yo")
                    eng.tensor_tensor(out=yo, in0=yw, in1=lnb_sb, op=Alu.add)
                else:
                    yo = yn
                nc.sync.dma_start(out=tens["out"][q * 128:(q + 1) * 128, :],
                                  in_=yo)

        # ---- attention head emitters ----
        head_pairs = {}

        def epilogue_q(h, otsb_tile, col0, q):
            tp = pps.tile([128, HD + 1], bf16, name="tp", tag="pps")
            nc.tensor.transpose(tp, otsb_tile[:, col0:col0 + 128], ident65)
            rec = small.tile([128, 1], f32, name="rec", tag="rec")
            nc.vector.reciprocal(out=rec, in_=tp[:, 0:1])
            nc.vector.scalar_tensor_tensor(
                out=oacc[q][:, h * HD:(h + 1) * HD],
                in0=tp[:, 1:HD + 1], scalar=rec, op0=Alu.mult,
                in1=xres_sb[:, q, h * HD:(h + 1) * HD], op1=Alu.add,
                accum_out=rs_all[:, q, h:h + 1])
            if h == H - 1:
                # sum of squares for LayerNorm variance (E[y^2] - mean^2).
                # Early chunks square on DVE (y*y via stt + accum) so the
                # final chunk's ACT chain (its 2 squares + sqrt) is short —
                # ACT's in-order queue would otherwise make the last chunk
                # wait behind all earlier squares.
                sqs = outp.tile([128, D], f32, name="sqs", tag="sqs")
                nc.scalar.activation(out=sqs, in_=oacc[q],
                                     func=Act.Square,
                                     accum_out=ssq8[:, q:q + 1])

        def emit_score_tile(h, k, pairs):
            """Scores for one k-tile + engine-split exp into pair tile."""
            blk = h // 2
            st = stp.tile([128, Q], f32, name="st", tag="st")
            for qcn in range(Q // 512):
                nc.tensor.matmul(
                    st[:, qcn * 512:(qcn + 1) * 512],
                    kt_sb[blk][:, k * 128:(k + 1) * 128],
                    qt_pad[h][:, qcn * 512:(qcn + 1) * 512],
                    start=None, stop=None)
            if k % 2 == 0:
                pairs.append(expp.tile([128, 2, Q], f8, name="ppair",
                                       tag="ppair"))
            pt = pairs[k // 2]
            if k not in DVE_EXP[h]:
                nc.scalar.activation(out=pt[:, k % 2, :], in_=st,
                                     func=Act.Exp,
                                     bias=maska_sb[:, k:k + 1], scale=SCALE)
            else:
                nc.vector.tensor_scalar(
                    out=pt[:, k % 2, :].bitcast(u8), in0=st,
                    scalar1=float(SCALE * SCHRAU_A),
                    scalar2=maskd_sb[:, k:k + 1],
                    op0=Alu.mult, op1=Alu.add)

        def emit_scores(h):
            pairs = head_pairs[h] = []
            for k in range(KT):
                emit_score_tile(h, k, pairs)

        def av_pair(h, ots, kp, qcn_range=(0, 1)):
            pairs = head_pairs[h]
            for qcn in qcn_range:
                nc.tensor.matmul(
                    ots[qcn], v8_sb[kp][:, :, h, 0:HD + 1],
                    pairs[kp][:, :, qcn * 512:(qcn + 1) * 512],
                    start=(kp == 0), stop=(kp == KP - 1),
                    perf_mode=DR)

        def emit_av(h, inter_with=None, ots=None, done_pairs=0, extra=()):
            pairs = head_pairs[h]
            extra = list(extra)
            # O^T[1+d, q] accumulated over k-pairs via fp8 DoubleRow; V_h
            # stationary so its weight load hides behind the 512-col moving
            # stream. Interleaved per k-pair with the NEXT head's score/exp
            # emission (and any deferred projection chunks) so ACT/PE never
            # starve behind a dense attention@V block.
            if ots is None:
                ots = [scr.tile([HD + 1, 512], f32, name=f"ot{qcn}", tag="ot")
                       for qcn in range(Q // 512)]
            if h != H - 1:
                otsb = [otsbp.tile([HD + 1, 512], bf16, name=f"otsb{qcn}",
                                   tag=f"otsb{qcn}") for qcn in range(Q // 512)]
                if inter_with is not None:
                    npairs = head_pairs[inter_with] = []
                for kp in range(done_pairs, KP):
                    av_pair(h, ots, kp)
                    if inter_with is not None:
                        emit_score_tile(inter_with, 2 * kp, npairs)
                        emit_score_tile(inter_with, 2 * kp + 1, npairs)
                    if extra:
                        extra.pop(0)()
                for qcn in range(Q // 512):
                    nc.vector.tensor_copy(out=otsb[qcn], in_=ots[qcn])
                for q in range(QT):
                    epilogue_q(h, otsb[q // 4], (q % 4) * 128, q)
            else:
                # last head: 4 chunks of 256 query columns; each chunk's 2
                # q-tiles run their epilogue + LayerNorm group immediately,
                # so only the final chunk's epilogue trails the last matmul
                for qg in range(4):
                    qcn, c0 = qg // 2, (qg % 2) * 256
                    for kp in range(KP):
                        nc.tensor.matmul(
                            ots[qcn][:, c0:c0 + 256],
                            v8_sb[kp][:, :, h, 0:HD + 1],
                            pairs[kp][:, :, qg * 256:(qg + 1) * 256],
                            start=(kp == 0), stop=(kp == KP - 1),
                            perf_mode=DR)
                    otsb = otsbp.tile([HD + 1, 256], bf16, name="otsbc",
                                      tag="otsbc")
                    nc.vector.tensor_copy(out=otsb,
                                          in_=ots[qcn][:, c0:c0 + 256])
                    for qi in range(2):
                        epilogue_q(h, otsb, qi * 128, qg * 2 + qi)
                    emit_ln_group(qg)

        # ---- emission. Block-0 projections interleave with head 0's first
        # score tiles so the exp stream starts as early as the DMA critical
        # path allows; V-proj and attention@V fill the PE queue behind it.
        kt_chunk(0, 0, on_act=True)
        qt_chunk(0, 0, on_act=True)
        qt_chunk(0, 1, on_act=True)
        kt_chunk(0, 1, on_act=True)
        pairs0 = head_pairs[0] = []
        ots0 = [scr.tile([HD + 1, 512], f32, name=f"ot{qcn}", tag="ot")
                for qcn in range(Q // 512)]
        for kp in range(KP):
            emit_score_tile(0, 2 * kp, pairs0)
            emit_score_tile(0, 2 * kp + 1, pairs0)
            if kp == 1:
                kt_chunk(0, 2, on_act=True)
            if kp == 2:
                kt_chunk(0, 3, on_act=True)
            emit_v(2 * kp, on_act=(kp < 4))
            emit_v(2 * kp + 1, on_act=(kp < 4))
            av_pair(0, ots0, kp)
        # head 1 scores standalone (ACT-bound stretch: deferred block-1
        # projections slot between score tiles without starving exp)
        from functools import partial
        blk1 = [partial(kt_chunk, 1, t) for t in range(4)] + [
            partial(qt_chunk, 1, c) for c in range(2)]
        pairs1 = head_pairs[1] = []
        for k in range(KT):
            emit_score_tile(1, k, pairs1)
            if k % 3 == 2 and blk1:
                blk1.pop(0)()
        while blk1:
            blk1.pop(0)()
        emit_av(0, ots=ots0, done_pairs=KP)
        blk2 = [partial(kt_chunk, 2, t) for t in range(4)] + [
            partial(qt_chunk, 2, c) for c in range(2)]
        emit_av(1, inter_with=2, extra=blk2[:3])
        emit_av(2, inter_with=3, extra=blk2[3:])
        blk3 = [partial(kt_chunk, 3, t) for t in range(4)] + [
            partial(qt_chunk, 3, c) for c in range(2)]
        emit_av(3, inter_with=4, extra=blk3[:3])
        emit_av(4, inter_with=5, extra=blk3[3:])
        emit_av(5, inter_with=6)
        emit_av(6, inter_with=7)
        emit_av(H - 1)

        # (residual + LayerNorm is emitted per chunk from the last head)


def _build(affine):
    import concourse.bacc as bacc
    import concourse.tile as tile
    from concourse import mybir

    f32 = mybir.dt.float32
    bf16 = mybir.dt.bfloat16
    nc = bacc.Bacc("TRN2", target_bir_lowering=False, debug=False)

    tens = {
        "xt": nc.dram_tensor("xt", [128, DC, T], bf16, kind="ExternalInput"),
        "xres": nc.dram_tensor("xres", [128, QT, D], f32,
                               kind="ExternalInput"),
        "wt": nc.dram_tensor("wt", [128, DC, 2 * D], bf16,
                             kind="ExternalInput"),
        "x8p": nc.dram_tensor("x8p", [2, 128, 2, T], mybir.dt.float8e4,
                              kind="ExternalInput"),
        "wv8": nc.dram_tensor("wv8", [2, 128, 2, D], mybir.dt.float8e4,
                              kind="ExternalInput"),
        "smalls": nc.dram_tensor("smalls", [128, 44], f32,
                                 kind="ExternalInput"),
        "out": nc.dram_tensor("out", [Q, D], f32, kind="ExternalOutput"),
    }
    if affine:
        tens["lnw"] = nc.dram_tensor("lnw", [D], f32, kind="ExternalInput")
        tens["lnb"] = nc.dram_tensor("lnb", [D], f32, kind="ExternalInput")

    with tile.TileContext(nc) as tc:
        _emit(nc, tc, tens, affine)
    nc.compile()
    return nc


def make_in_maps(query, key_mask, in_proj_weight, in_proj_bias, ln_weight,
                 ln_bias):
    import ml_dtypes

    bf = ml_dtypes.bfloat16
    query = np.asarray(query, dtype=np.float32)
    key_mask = np.asarray(key_mask)
    w = np.asarray(in_proj_weight, dtype=np.float32)
    b = np.asarray(in_proj_bias, dtype=np.float32)
    lnw = np.asarray(ln_weight, dtype=np.float32)
    lnb = np.asarray(ln_bias, dtype=np.float32)
    affine = not (np.all(lnw == 1.0) and np.all(lnb == 0.0))

    # wt host layout: [K_blk0 | Q_blk0 | K_blk1 | Q_blk1 | ...] 128-col
    # groups, rows regrouped [128, 4, cols] so the critical projection
    # weights arrive in one DMA issue. V columns live only in wv8.
    wcols = []
    for i in range(DC):
        wcols.append(w.T[:, D + 128 * i: D + 128 * (i + 1)])  # K block i
        wcols.append(w.T[:, 128 * i: 128 * (i + 1)])          # Q block i
    wtr = np.concatenate(wcols, axis=1).astype(bf)            # [512, 1024]
    wt = np.ascontiguousarray(wtr.reshape(DC, 128, 2 * D).transpose(1, 0, 2))
    btr = np.ascontiguousarray(b.reshape(12, 128).T)
    bv = b[2 * D:3 * D]  # folded into xres: attn-out(v+bv) = attn-out(v)+bv
    in_maps = []
    for c in range(NCORES):
        bi, half = c // 2, c % 2
        xb = query[bi]
        # k-columns reordered so this core's query half sits at 0:Q — the
        # Q-projection then reads xt directly (no separate xq input) and
        # attention is permutation-invariant over k as long as the mask
        # follows the same order.
        perm = (np.r_[Q:T, 0:Q] if half else np.arange(T))
        xbt = np.ascontiguousarray(
            xb.T[:, perm].astype(bf).reshape(DC, 128, T).transpose(1, 0, 2))
        # fp8 operands quantized straight from f32: rounding f32->bf16->fp8
        # instead costs 1.5x in final max-error (boundary double rounding)
        f8 = ml_dtypes.float8_e4m3
        xbt8 = xb.T[:, perm].astype(f8)
        x8p = np.ascontiguousarray(
            xbt8.reshape(2, 2, 128, T).transpose(0, 2, 1, 3))
        wv8 = np.ascontiguousarray(
            w[2 * D:3 * D].T.astype(f8)
            .reshape(2, 2, 128, D).transpose(0, 2, 1, 3))
        km = key_mask[bi][perm]
        maskb = np.where(km, np.float32(MASK_BIAS), np.float32(0.0))
        maska = (maskb - LNP).astype(np.float32).reshape(KT, 128).T
        maskd = np.where(km, np.float32(-1e6),
                         np.float32(SCHRAU_B)).reshape(KT, 128).T
        smalls = np.concatenate([btr, maska, maskd], axis=1).astype(np.float32)
        xres = (xb[half * Q:(half + 1) * Q] + bv[None, :]).astype(np.float32)
        im = {
            "xt": xbt,
            "xres": np.ascontiguousarray(
                xres.reshape(QT, 128, D).transpose(1, 0, 2)),
            "wt": wt,
            "x8p": x8p,
            "wv8": wv8,
            "smalls": np.ascontiguousarray(smalls),
        }
        if affine:
            im["lnw"] = lnw
            im["lnb"] = lnb
        in_maps.append(im)
    return in_maps


def assemble(results):
    out = np.empty((B, T, D), dtype=np.float32)
    for c in range(NCORES):
        bi, half = c // 2, c % 2
        out[bi, half * Q:(half + 1) * Q] = results[c]["out"]
    return out


def get_nc(affine=False):
    key = ("nc", affine)
    if key not in _CACHE:
        _CACHE[key] = _build(affine)
    return _CACHE[key]


def kernel(query, key_mask, in_proj_weight, in_proj_bias, ln_weight, ln_bias):
    from concourse.bass_utils import run_bass_kernel_spmd

    affine = not (np.all(np.asarray(ln_weight) == 1.0)
                  and np.all(np.asarray(ln_bias) == 0.0))
    nc = get_nc(affine)
    in_maps = make_in_maps(query, key_mask, in_proj_weight, in_proj_bias,
                           ln_weight, ln_bias)
    res = run_bass_kernel_spmd(nc, in_maps, core_ids=list(range(NCORES)))
    return assemble(res.results)
